# revision 26
# baseline (speedup 1.0000x reference)
"""Trainium2 Bass kernel for nn_Decoder_14894946583396 (dense_mlp).

Reference computation:
    sized = broadcast(representation[B,1,R] -> [B,S,R])   (ones @ size_matrix-shaped ones)
    h     = relu(sized @ W1^T + b1)                       [B,S,HID]
    out   = h @ W2^T + b2                                 [B,S,OUT]

Because every position s within batch b receives the identical input row
representation[b], the MLP output row is identical for all S positions:
    row[b] = relu(rep[b] @ W1^T + b1) @ W2^T + b2         [B,OUT]
    out[b, s, :] = row[b]  for all s

Data-parallel across 8 NeuronCores: 4 batches per core, replicated
weights. The per-core kernel computes the tiny MLP in fp16 on the
TensorEngine (1 cycle/row vs fp32's 4, and half the weight-load bytes)
and broadcast-writes each row across S with stride-0-source SBUF->DRAM
DMAs. Measured phases: ~7.5us framework preamble, inputs stream
~390 GB/s on one ring, first output store fires ~21us, stores sustain
400-425 GB/s (the per-core SDMA/fabric ceiling), ~2.3us drain tail.

Device pipeline per core:
  1. ALL inputs live in one packed fp16 DRAM tensor, streamed over the
     sync HWDGE ring as 4 chunked DMAs in consumption order:
       A: x^T + I4 + ones rows + W1h0   (so L1 starts on arrival)
       B: W1h1 + bias block (ones@p0, b1@p32, b2h0@p64, b2h1@p0)
       C0: W2 for output half 0
       C1: W2 for output half 1 + selector blocks
     A single ring matters: spreading inputs over 2-3 queues made the
     SDMA engines round-robin between rings and HALVED aggregate read
     bandwidth. 4 input + 4 output DMAs also fill the 8 DMAHW
     semaphore lanes exactly; a 9th DMA would recycle a lane and add a
     second sync wait on the reusing trigger, which this walrus
     rejects.
  2. Two fp32 warmup matmuls on zeros (4 cycles/row = high sustained
     activity) run under the input stream and trip the HAM clock ramp
     (1.2 -> 2.4 GHz) right as L1 starts; an all-fp16 instruction
     stream alone was never granted the fast clock.
  3. L1: H[m,h] = x @ W1^T via 8 accumulating fp16 matmuls (x^T chunk
     stationary, W1 chunk moving) + a K=1 ones-matmul folding in b1;
     relu casts to fp16 on ScalarE.
  4. H -> H^T via 4 fp16 PE transposes, copied to SBUF on DVE.
  5. L2 per 512-col half: Y = H @ W2^T (4 fp16 matmuls) + K=1 bias
     matmul; PSUM -> SBUF fp16 cast (DVE half 0, ACT half 1).
  6. Broadcast per batch/half: K=4 selector matmul (lhsT = e_b x ones,
     host-packed) replicates row b across all 128 partitions in one PE
     op; PSUM -> SBUF f32 copy (DVE even b, ACT odd b).
  7. Stores read the [128,1024] broadcast tiles with a stride-0
     repeat dim (each partition's 4 KB row is read 8x; descriptor size
     does not matter, 4 KB descriptors already sustain the plateau).
     Batch 0 gates the write phase, so it ships per 512-col half the
     moment each broadcast copy lands: h0 on the sync HWDGE ring, h1
     on SWDGE whose separate DMASW semaphore pool keeps the HWDGE
     budget at 8. Batches 1-3 are one 4 MiB DMA each, alternating the
     sync/scalar rings. Splitting W1 into 3 stream chunks and issuing
     b0's first-half selector before L2's second half were both tried
     and measured as neutral-to-negative; this layout won the pooled
     A/B.

The Tile layer auto-inserts single-wait sync NOPs where an instruction
would need 2+ semaphore waits; the explicit nop chain before the
TileContext exit keeps the final drain itself at <=1 wait.

fp16 numerics: worst-case relative error vs the f32 reference is
~4.5e-4 (tolerance 2e-2) — x, W1, W2, b1, b2 round to fp16 but all
accumulation is f32 in PSUM and the broadcast/store path is f32.
"""
import sys

import numpy as np

if "/opt/trn_rl_repo" not in sys.path:
    sys.path.insert(0, "/opt/trn_rl_repo")

B, S, R = 32, 1024, 1024
HID, OUT = 512, 1024
N_CORES = 8
BPC = B // N_CORES  # batches per core

RC = R // 128  # layer-1 contraction chunks
HC = HID // 128  # layer-2 contraction chunks
OC = OUT // 512  # 512-wide output column chunks
SC = S // 128  # broadcast repeats per output DMA

# pin columns (fp16), in DMA-chunk order:
#   A: x^T (32) | I4+ones (4) | W1h0 (2048)
#   B: W1h1 (2048) | bias block (512: b1@p32, b2h0@p64, b2h1@p96)
#   C: W2 oc0 (2048) | W2 oc1 (2048)
#   D: selector blocks (512, rows 0..3)
XTOFF = 0
I4OFF = XTOFF + RC * BPC  # 32
W1OFF = I4OFF + BPC  # 36
BIASOFF = W1OFF + RC * HID  # 4132, first 4 cols: ones row at p0
BVAL = BIASOFF + 4  # bias values: b1@p32, b2h0@p64, b2h1@p0
W2OFF = BVAL + 512  # 4648
SELOFF = W2OFF + OC * HC * 512  # 8740
PINW = SELOFF + BPC * 128  # 9252
AEND = W1OFF + RC * HID // 2  # 2084
BEND = W2OFF

_CACHED_NC = None


def _build_nc():
    import concourse.bass as bass
    import concourse.mybir as mybir
    from concourse.tile import TileContext, add_dep_helper

    f32 = mybir.dt.float32
    f16 = mybir.dt.float16
    relu = mybir.ActivationFunctionType.Relu
    nc = bass.Bass()

    pin = nc.dram_tensor("pin", [128, PINW], f16, kind="ExternalInput")
    out = nc.dram_tensor("out", [BPC, S, OUT], f32, kind="ExternalOutput")

    with TileContext(nc) as tc:
        with (
            tc.tile_pool(name="const", bufs=1) as cpool,
            tc.tile_pool(name="psum_s", bufs=1, space="PSUM") as pp_s,
            tc.tile_pool(name="psum_y", bufs=2, space="PSUM") as pp_y,
            tc.tile_pool(name="psum_t", bufs=2, space="PSUM") as pp_t,
            tc.tile_pool(name="psum_bc", bufs=3, space="PSUM") as pp_bc,
        ):
            p = cpool.tile([128, PINW], f16, tag="pin")
            # 4 input + 4 output DMAs = the 8 DMA semaphore lanes exactly;
            # a 9th DMA would recycle a lane and add a second sync wait on
            # the reusing trigger, which this walrus rejects
            # 3 input chunks (w2 halves + selectors merged) so all 5 output
            # stores fit the 8 HWDGE semaphore lanes with NO SWDGE store —
            # testing whether the E79 tail-straggler erratum (SWDGE
            # descriptor-ring port contention) disappears
            chunks = [0, AEND, BEND, PINW]
            in_dmas = []
            for i in range(len(chunks) - 1):
                d = nc.sync.dma_start(
                    out=p[:, chunks[i] : chunks[i + 1]],
                    in_=pin[:, chunks[i] : chunks[i + 1]],
                )
                in_dmas.append(d)

            # ---- PE warmup: two fp32 matmuls on zeros (4 cycles/row = high
            # sustained activity) force the HAM clock ramp (1.2 -> 2.4 GHz)
            # during the otherwise-idle input-DMA window; fp16 L1 matmuls
            # alone never trip the ramp threshold ------------------------
            wm_sb = cpool.tile([128, 512], f32, tag="wm")
            nc.vector.memset(wm_sb[:, :], 0.0)
            ph_full = pp_s.tile([128, HID], f32, tag="s")
            for k in range(2):
                nc.tensor.matmul(
                    ph_full[:, :],
                    lhsT=wm_sb[:, 0:128],
                    rhs=wm_sb[:, :],
                    start=True,
                    stop=True,
                )

            # ---- L1: H[m, h] = x @ W1^T + b1, relu -------------------------
            ph = ph_full[0:BPC, :]
            for rc in range(RC):
                nc.tensor.matmul(
                    ph[:, :],
                    lhsT=p[:, XTOFF + rc * BPC : XTOFF + (rc + 1) * BPC],
                    rhs=p[:, W1OFF + rc * HID : W1OFF + rc * HID + HID],
                    start=(rc == 0),
                    stop=False,
                )
            nc.tensor.matmul(
                ph[:, :],
                lhsT=p[32:33, I4OFF : I4OFF + BPC],
                rhs=p[32:33, BVAL : BVAL + HID],
                start=False,
                stop=True,
            )
            h_sb = cpool.tile([BPC, HID], f16, tag="h")
            nc.scalar.activation(h_sb[:, :], ph[:, :], relu)

            # ---- H -> H^T (fp16, stationary operand for L2) ----------------
            ht_sb = cpool.tile([128, HC * BPC], f16, tag="ht")
            for hc in range(HC):
                pt = pp_t.tile([128, BPC], f16, tag="t")
                nc.tensor.transpose(
                    pt[:, :],
                    h_sb[0:BPC, hc * 128 : (hc + 1) * 128],
                    p[0:BPC, I4OFF : I4OFF + BPC],
                )
                nc.vector.tensor_copy(ht_sb[:, hc * BPC : (hc + 1) * BPC], pt[:, :])

            # ---- L2: Y[m, o] = H @ W2^T + b2, fp16 y rows ------------------
            y_halves = []
            for oc in range(OC):
                py = pp_y.tile([BPC, 512], f32, tag="y")
                for hc in range(HC):
                    w2c = W2OFF + oc * HC * 512 + hc * 512
                    nc.tensor.matmul(
                        py[:, :],
                        lhsT=ht_sb[:, hc * BPC : (hc + 1) * BPC],
                        rhs=p[:, w2c : w2c + 512],
                        start=(hc == 0),
                        stop=False,
                    )
                bp = 64 if oc == 0 else 0
                ones_c = I4OFF if oc == 0 else BIASOFF
                nc.tensor.matmul(
                    py[:, :],
                    lhsT=p[bp : bp + 1, ones_c : ones_c + BPC],
                    rhs=p[bp : bp + 1, BVAL : BVAL + 512],
                    start=False,
                    stop=True,
                )
                yh = cpool.tile([BPC, 512], f16, tag=f"yh{oc}")
                if oc == 0:
                    nc.vector.tensor_copy(yh[:, :], py[:, :])
                else:
                    nc.scalar.activation(
                        yh[:, :], py[:, :], mybir.ActivationFunctionType.Copy
                    )
                y_halves.append(yh)

            # ---- broadcast rows across partitions, store -------------------
            # A K=4 selector matmul (lhsT = e_b outer ones, host-packed)
            # extracts row b of Y AND replicates it across all 128 output
            # partitions in one PE op. One 4 MiB DMA per batch reads the
            # [128,1024] tile 8x via a stride-0 dim.
            # ---- broadcast rows across partitions, store -------------------
            # A K=4 selector matmul (lhsT = e_b outer ones, host-packed)
            # extracts row b of Y AND replicates it across all 128 output
            # partitions in one PE op. One 4 MiB DMA per batch reads the
            # [128,1024] tile 8x via a stride-0 dim.
            out_dmas = []
            last_act = None
            last_dve = None
            for b in range(BPC):
                ybc = cpool.tile([128, OUT], f32, tag=f"ybc{b}")
                for oc in range(OC):
                    pb = pp_bc.tile([128, 512], f32, tag="bc")
                    last_mm = nc.tensor.matmul(
                        pb[:, :],
                        lhsT=p[0:BPC, SELOFF + b * 128 : SELOFF + (b + 1) * 128],
                        rhs=y_halves[oc][0:BPC, :],
                        start=True,
                        stop=True,
                    )
                    dst = ybc[:, oc * 512 : (oc + 1) * 512]
                    if b % 2 == 0:
                        last_dve = nc.vector.tensor_copy(dst, pb[:, :])
                    else:
                        last_act = nc.scalar.activation(
                            dst, pb[:, :], mybir.ActivationFunctionType.Copy
                        )
                    if b == 0:
                        # b0 gates the write phase: ship each 512-col half as
                        # soon as its broadcast copy lands. h0 rides the sync
                        # HWDGE ring; h1 uses SWDGE, whose separate DMASW
                        # semaphore pool keeps the HWDGE lane budget at 8.
                        dma_eng = nc.sync if oc == 0 else nc.scalar
                        d = dma_eng.dma_start(
                            out=out[0].rearrange("(c p) o -> p c o", c=SC)[
                                :, :, oc * 512 : (oc + 1) * 512
                            ],
                            in_=ybc[:, oc * 512 : (oc + 1) * 512]
                            .unsqueeze(1)
                            .broadcast_to((128, SC, 512)),
                        )
                        out_dmas.append(d)
                if b > 0:
                    dma_eng = nc.sync if b % 2 == 0 else nc.scalar
                    d = dma_eng.dma_start(
                        out=out[b].rearrange("(c p) o -> p c o", c=SC),
                        in_=ybc[:, :].unsqueeze(1).broadcast_to((128, SC, OUT)),
                    )
                    out_dmas.append(d)

            # The kernel-tail drain waits on every proc's final tick, but this
            # walrus allows at most ONE sync wait per instruction. Chain SP
            # nops, one dependency each, so SP's vector clock observes the
            # final tick of every DMA lane and engine before the drain.
            tail = out_dmas + in_dmas + [last_mm, last_act, last_dve]
            for d in tail:
                n = nc.sync.nop(nofuse=True)
                add_dep_helper(
                    n.ins, d.ins, sync=True, reason="observe final ticks pre-drain"
                )

    return nc


def _get_nc():
    global _CACHED_NC
    if _CACHED_NC is None:
        _CACHED_NC = _build_nc()
    return _CACHED_NC


def _prep_in_maps(representation, W1, b1, W2, b2):
    rep = np.asarray(representation, dtype=np.float32).reshape(B, R)
    w1 = np.asarray(W1, dtype=np.float32)
    w2 = np.asarray(W2, dtype=np.float32)
    b1 = np.asarray(b1, dtype=np.float32)
    b2 = np.asarray(b2, dtype=np.float32)

    # w1p[p, rc*HID + h] = W1[h, rc*128 + p]
    w1p = np.ascontiguousarray(
        w1.T.reshape(RC, 128, HID).transpose(1, 0, 2).reshape(128, RC * HID)
    ).astype(np.float16)
    # w2p[p, oc*HC*512 + hc*512 + j] = W2[oc*512 + j, hc*128 + p]
    w2p = (
        w2.reshape(OC, 512, HC, 128)
        .transpose(3, 0, 2, 1)
        .reshape(128, OC * HC * 512)
    )
    w2p = np.ascontiguousarray(w2p).astype(np.float16)

    in_maps = []
    for c in range(N_CORES):
        xt = rep[c * BPC : (c + 1) * BPC].T  # [R, BPC]
        pin = np.zeros((128, PINW), dtype=np.float16)
        pin[:, XTOFF : XTOFF + RC * BPC] = (
            xt.reshape(RC, 128, BPC).transpose(1, 0, 2).reshape(128, RC * BPC)
        ).astype(np.float16)
        pin[0:BPC, I4OFF : I4OFF + BPC] = np.eye(BPC, dtype=np.float16)
        for q in (32, 64):
            pin[q, I4OFF : I4OFF + BPC] = 1.0
        pin[0, BIASOFF : BIASOFF + BPC] = 1.0
        pin[:, W1OFF : W1OFF + RC * HID] = w1p
        pin[32, BVAL : BVAL + HID] = b1.astype(np.float16)
        pin[64, BVAL : BVAL + 512] = b2[0:512].astype(np.float16)
        pin[0, BVAL : BVAL + 512] = b2[512:1024].astype(np.float16)
        pin[:, W2OFF : W2OFF + OC * HC * 512] = w2p
        for b in range(BPC):
            pin[b, SELOFF + b * 128 : SELOFF + (b + 1) * 128] = 1.0
        in_maps.append({"pin": pin})
    return in_maps


def run_sharded(representation, W1, b1, W2, b2, **run_kwargs):
    """Compile+run on 8 cores; returns (full_output, BassKernelResults)."""
    from concourse.bass_utils import run_bass_kernel_spmd

    nc = _get_nc()
    in_maps = _prep_in_maps(representation, W1, b1, W2, b2)
    res = run_bass_kernel_spmd(nc, in_maps, core_ids=list(range(N_CORES)), **run_kwargs)
    full = np.concatenate([r["out"] for r in res.results], axis=0)
    return full, res


def kernel(representation, size_matrix=None, W1=None, b1=None, W2=None, b2=None):
    # size_matrix only contributes its shape in the reference (ones_like);
    # its values are unused.
    full, _ = run_sharded(representation, W1, b1, W2, b2)
    return full
